# revision 9
# baseline (speedup 1.0000x reference)
"""nn_Linear8bit on 8 TRN2 NeuronCores — column-parallel (tensor-parallel on out_features).

out[m, n] = sum_k x[m, k] * wq[n, k] * scale[n] + bias[n]
  x: [2, 512, 4096] f32, wq: [16384, 4096] int32 (int8-valued), scale/bias: [16384] f32

Sharding: W/scale/bias row-sharded 2048/core; x replicated (fed k-major). No collectives.

Host prep (pure layout/bit repack, no arithmetic):
  - x -> x.T [K, M] f32 (k-major replica).
  - wq (int8-valued int32) -> int8, transposed+swizzled to [nt*128, kt, n] so each
    n-tile's stationary block DMAs as contiguous partition lines.
  - scale/bias -> [128, 16] (partition-major per n-tile).

Per-core dataflow (all HWDGE, no on-chip transposes):
  - x: f32 DMA on the ACT HWDGE ring in 1-k-tile pieces (first k-tile split into
    two 512-token halves so the first matmul can start ~11us in) -> DVE cast
    f32->bf16 into resident xT tiles (contraction on partitions).
  - W: int8 DMA on the SP HWDGE ring; first 4 n-tiles split (kt 0..7 / 8..31).
    The small kt0..7 pieces cast on DVE (interleaved with the first x casts);
    everything else casts int8->bf16 on the otherwise-idle GpSimd engine so the
    in-order DVE queue never delays an x cast.
  - 7 dummy warm-up matmuls on a memset tile run during the initial DMA dead
    time so the PE_HAM clock-gate is at 8/8 when real matmuls start.
  - Startup phase: first 4 n-tiles processed k-major with 8 live PSUM
    accumulators while x streams in; steady phase: remaining 12 n-tiles k-inner,
    ch-inner (one stationary per (nt,kt) feeds both 512-token chunks).
  - PSUM evicted via one DVE tensor_scalar (x*scale + bias, per-partition
    scalars); outputs stored as out.T f32 on the SP ring.
  - host: concat core outputs along n, transpose to [1024, 16384].
"""

import numpy as np

import concourse.tile as tile
from concourse import bacc, mybir
from concourse.bass_utils import run_bass_kernel_spmd

B, S, K, N = 2, 512, 4096, 16384
M = B * S              # 1024 tokens
NCORES = 8
NSH = N // NCORES      # 2048 out-features per core
P = 128
KT = K // P            # 32 k-tiles
NT = NSH // P          # 16 n-tiles per core
MCW = 512              # moving free dim per matmul (= one PSUM bank of f32)
MCH = M // MCW         # 2 token chunks
NT_A = 4               # n-tiles processed in the startup phase
WSPL = 8               # startup W tiles split at this k-tile
NDUMMY = 7             # warm-up matmuls


def build(w_bufs: int = 4, x_bufs: int = 6, psum_bufs: int = 8):
    nc = bacc.Bacc("TRN2", target_bir_lowering=False, debug=False)
    xT_d = nc.dram_tensor("xT", [K, M], mybir.dt.float32, kind="ExternalInput")
    w_d = nc.dram_tensor("wq", [NT * P, KT, P], mybir.dt.int8, kind="ExternalInput")
    s_d = nc.dram_tensor("scale", [P, NT], mybir.dt.float32, kind="ExternalInput")
    b_d = nc.dram_tensor("bias", [P, NT], mybir.dt.float32, kind="ExternalInput")
    o_d = nc.dram_tensor("outT", [NSH, M], mybir.dt.float32, kind="ExternalOutput")

    with tile.TileContext(nc) as tc:
        with (
            tc.tile_pool(name="xT_pool", bufs=1) as xT_pool,
            tc.tile_pool(name="xstage", bufs=x_bufs) as xstage_pool,
            tc.tile_pool(name="x0stage", bufs=1) as x0stage_pool,
            tc.tile_pool(name="w8", bufs=w_bufs) as w8_pool,
            tc.tile_pool(name="w8ab", bufs=1) as w8ab_pool,
            tc.tile_pool(name="wT_pool", bufs=w_bufs) as wT_pool,
            tc.tile_pool(name="wTab", bufs=1) as wTab_pool,
            tc.tile_pool(name="small", bufs=2) as small_pool,
            tc.tile_pool(name="osb", bufs=4) as osb_pool,
            tc.tile_pool(name="psum", bufs=psum_bufs, space="PSUM") as psum_pool,
        ):
            # ---- PE warm-up: dummy matmuls on a zeroed tile during DMA dead time
            dummy = small_pool.tile([P, MCW], mybir.dt.bfloat16, tag="dummy")
            nc.vector.memset(dummy[:], 0.0)

            psA = [
                [
                    psum_pool.tile(
                        [P, MCW], mybir.dt.float32, name=f"psA{nt}_{c}", tag="ps"
                    )
                    for c in range(MCH)
                ]
                for nt in range(NT_A)
            ]
            for i in range(NDUMMY):
                nc.tensor.matmul(
                    psA[0][0][:], dummy[:, 0:P], dummy[:], start=True, stop=True
                )

            # ---- startup DMAs.
            # SP ring: W kt0..WSPL pieces for the first NT_A n-tiles, then the
            # remainders.  ACT ring: x, 1 k-tile at a time (kt0 in two halves).
            w8a, w8b = {}, {}
            for nt in range(NT_A):
                w8a[nt] = w8ab_pool.tile(
                    [P, WSPL, P], mybir.dt.int8, name=f"w8a{nt}", tag=f"w8a{nt}"
                )
                nc.sync.dma_start(
                    out=w8a[nt][:], in_=w_d.ap()[nt * P:(nt + 1) * P, 0:WSPL]
                )
            for nt in range(NT_A):
                w8b[nt] = w8ab_pool.tile(
                    [P, KT - WSPL, P], mybir.dt.int8, name=f"w8b{nt}",
                    tag=f"w8b{nt}"
                )
                nc.sync.dma_start(
                    out=w8b[nt][:], in_=w_d.ap()[nt * P:(nt + 1) * P, WSPL:KT]
                )

            x0stg = []
            for h in range(MCH):
                stg = x0stage_pool.tile(
                    [P, 1, MCW], mybir.dt.float32, name=f"x0stg{h}", tag=f"x0stg{h}"
                )
                nc.scalar.dma_start(
                    out=stg[:],
                    in_=xT_d.ap()[0:P, h * MCW:(h + 1) * MCW].rearrange(
                        "(kt p) m -> p kt m", p=P
                    ),
                )
                x0stg.append(stg)
            xstgs = [None] * KT
            for kt in range(1, KT):
                xstg = xstage_pool.tile(
                    [P, 1, M], mybir.dt.float32, name=f"xstg{kt}", tag="xstg"
                )
                nc.scalar.dma_start(
                    out=xstg[:],
                    in_=xT_d.ap()[kt * P:(kt + 1) * P, :].rearrange(
                        "(kt p) m -> p kt m", p=P
                    ),
                )
                xstgs[kt] = xstg
            s_sb = small_pool.tile([P, NT], mybir.dt.float32, tag="s_sb")
            nc.sync.dma_start(out=s_sb[:], in_=s_d.ap())
            b_sb = small_pool.tile([P, NT], mybir.dt.float32, tag="b_sb")
            nc.sync.dma_start(out=b_sb[:], in_=b_d.ap())

            # ---- DVE cast order: W a-pieces and earliest x pieces interleaved.
            wTa, wTb = {}, {}
            xT0 = [None] * MCH
            xTs = [None] * KT

            def cast_x0(h):
                xt = xT_pool.tile(
                    [P, 1, MCW], mybir.dt.bfloat16, name=f"xT0{h}", tag=f"xT0{h}"
                )
                nc.vector.tensor_copy(out=xt[:], in_=x0stg[h][:])
                xT0[h] = xt

            def cast_x(kt):
                xt = xT_pool.tile(
                    [P, 1, M], mybir.dt.bfloat16, name=f"xT{kt}", tag=f"xT{kt}"
                )
                nc.vector.tensor_copy(out=xt[:], in_=xstgs[kt][:])
                xTs[kt] = xt

            def cast_wa(nt):
                wTa[nt] = wTab_pool.tile(
                    [P, WSPL, P], mybir.dt.bfloat16, name=f"wTa{nt}", tag=f"wTa{nt}"
                )
                nc.vector.tensor_copy(out=wTa[nt][:], in_=w8a[nt][:])

            cast_wa(0)
            cast_x0(0)
            cast_wa(1)
            cast_x0(1)
            cast_wa(2)
            cast_x(1)
            cast_wa(3)
            for kt in range(2, KT):
                cast_x(kt)

            # ---- GpSimd cast queue: W remainders, then phase-B prefetch tiles.
            for nt in range(NT_A):
                wTb[nt] = wTab_pool.tile(
                    [P, KT - WSPL, P], mybir.dt.bfloat16, name=f"wTb{nt}",
                    tag=f"wTb{nt}"
                )
                nc.gpsimd.tensor_copy(out=wTb[nt][:], in_=w8b[nt][:])

            wTs = {}

            def load_w_full(nt):
                w8 = w8_pool.tile([P, KT, P], mybir.dt.int8, name=f"w8_{nt}", tag="w8")
                nc.sync.dma_start(out=w8[:], in_=w_d.ap()[nt * P:(nt + 1) * P])
                wT = wT_pool.tile(
                    [P, KT, P], mybir.dt.bfloat16, name=f"wT{nt}", tag="wT"
                )
                nc.gpsimd.tensor_copy(out=wT[:], in_=w8[:])
                wTs[nt] = wT

            for nt in range(NT_A, min(NT_A + 4, NT)):
                load_w_full(nt)

            def stationary(nt, kt):
                if nt < NT_A:
                    if kt < WSPL:
                        return wTa[nt][:, kt, :]
                    return wTb[nt][:, kt - WSPL, :]
                return wTs[nt][:, kt, :]

            def moving(kt, c):
                if kt == 0:
                    return xT0[c][:, 0, :]
                return xTs[kt][:, 0, c * MCW:(c + 1) * MCW]

            # ---- phase A matmuls: k-major across NT_A n-tiles.
            # kt0 runs c-outer so the second m-half's cast can trail the first.
            for c in range(MCH):
                for nt in range(NT_A):
                    nc.tensor.matmul(
                        psA[nt][c][:], stationary(nt, 0), moving(0, c),
                        start=True, stop=False,
                    )
            for kt in range(1, KT):
                for nt in range(NT_A):
                    for c in range(MCH):
                        nc.tensor.matmul(
                            psA[nt][c][:], stationary(nt, kt), moving(kt, c),
                            start=False, stop=(kt == KT - 1),
                        )

            def evict(nt, c, ps):
                o_sb = osb_pool.tile(
                    [P, MCW], mybir.dt.float32, name=f"osb{nt}_{c}", tag="o_sb"
                )
                nc.vector.tensor_scalar(
                    out=o_sb[:],
                    in0=ps[:],
                    scalar1=s_sb[:, nt:nt + 1],
                    scalar2=b_sb[:, nt:nt + 1],
                    op0=mybir.AluOpType.mult,
                    op1=mybir.AluOpType.add,
                )
                nc.sync.dma_start(
                    out=o_d.ap()[nt * P:(nt + 1) * P, c * MCW:(c + 1) * MCW],
                    in_=o_sb[:],
                )

            for nt in range(NT_A):
                for c in range(MCH):
                    evict(nt, c, psA[nt][c])

            # ---- phase B: remaining n-tiles, k-inner / ch-inner
            for nt in range(NT_A, NT):
                ps = [
                    psum_pool.tile(
                        [P, MCW], mybir.dt.float32, name=f"ps{nt}_{c}", tag="ps"
                    )
                    for c in range(MCH)
                ]
                for kt in range(KT):
                    for c in range(MCH):
                        nc.tensor.matmul(
                            ps[c][:], stationary(nt, kt), moving(kt, c),
                            start=(kt == 0), stop=(kt == KT - 1),
                        )
                if nt + 4 < NT:
                    load_w_full(nt + 4)
                for c in range(MCH):
                    evict(nt, c, ps[c])

    nc.compile()
    return nc


def make_in_maps(x, weight_quant, scale, bias):
    x2T = np.ascontiguousarray(
        np.asarray(x, dtype=np.float32).reshape(M, K).T
    )  # [K, M] k-major replica
    wq = np.asarray(weight_quant, dtype=np.int32).astype(np.int8)  # int8-valued
    scale = np.asarray(scale, dtype=np.float32)
    bias = np.asarray(bias, dtype=np.float32)
    in_maps = []
    for i in range(NCORES):
        sl = slice(i * NSH, (i + 1) * NSH)
        # [nsh, k] -> [nt, n, kt, p] -> [nt, p, kt, n] -> [nt*p, kt, n]
        w_sw = np.ascontiguousarray(
            wq[sl].reshape(NT, P, KT, P).transpose(0, 3, 2, 1)
        ).reshape(NT * P, KT, P)
        in_maps.append({
            "xT": x2T,
            "wq": w_sw,
            "scale": np.ascontiguousarray(scale[sl].reshape(NT, P).T),
            "bias": np.ascontiguousarray(bias[sl].reshape(NT, P).T),
        })
    return in_maps


def gather_output(results):
    outT = np.concatenate([np.asarray(r["outT"]) for r in results], axis=0)  # [N, M]
    return np.ascontiguousarray(outT.T).reshape(B, S, N).astype(np.float32, copy=False)


def kernel(x, weight_quant, scale, bias):
    nc = build()
    in_maps = make_in_maps(x, weight_quant, scale, bias)
    res = run_bass_kernel_spmd(nc, in_maps, core_ids=list(range(NCORES)))
    return gather_output(res.results)


if __name__ == "__main__":
    rng = np.random.default_rng(0)
    x = rng.standard_normal((B, S, K), dtype=np.float32)
    wq = rng.integers(-128, 128, size=(N, K), dtype=np.int64).astype(np.int32)
    scale = rng.uniform(0.001, 0.02, size=(N,)).astype(np.float32)
    bias = rng.standard_normal((N,), dtype=np.float32)
    out = kernel(x=x, weight_quant=wq, scale=scale, bias=bias)
    w = wq.astype(np.float32) * scale[:, None]
    exp = x.reshape(M, K) @ w.T + bias
    err = np.abs(out.reshape(M, N) - exp).max() / np.abs(exp).max()
    print("self-check rel err:", err)


# revision 10
# speedup vs baseline: 1.2564x; 1.2564x over previous
"""nn_Linear8bit on 8 TRN2 NeuronCores — column-parallel (tensor-parallel on out_features).

out[m, n] = sum_k x[m, k] * wq[n, k] * scale[n] + bias[n]
  x: [2, 512, 4096] f32, wq: [16384, 4096] int32 (int8-valued), scale/bias: [16384] f32

Sharding: W/scale/bias row-sharded 2048/core; x replicated (fed k-major). No collectives.

Host prep (pure layout/bit repack, no arithmetic):
  - x -> x.T [K, M] f32 (k-major replica).
  - wq (int8-valued int32) -> int8, transposed+swizzled to [nt*128, kt, n] so each
    n-tile's stationary block DMAs as contiguous partition lines.
  - scale/bias -> [128, 16] (partition-major per n-tile).

Per-core dataflow (all HWDGE, all casts on DVE, no on-chip transposes):
  - x: f32 DMA on the ACT HWDGE ring — kt0 in two 512-token halves, kt1 alone,
    then 2-k-tile pieces — so the first matmul starts ~11.5us in and arrivals
    stay smooth.  DVE casts f32->bf16 into resident xT tiles.
  - W: int8 DMA on the SP HWDGE ring; first 4 n-tiles split (kt 0..7 / 8..31)
    into separate tiles.  All W casts int8->bf16 run on DVE, interleaved into
    the x-cast chain at points where the PE won't need the result yet (the
    in-order DVE queue must never delay an x cast past its matmul).  GpSimd is
    deliberately unused: its cast ucode is ~25x slower and stalls concurrent
    DVE ops via SBUF port contention (measured).
  - 7 dummy warm-up matmuls on a memset tile run during the initial DMA dead
    time so the PE_HAM clock-gate is at 8/8 when real matmuls start.
  - Startup phase: first 4 n-tiles processed k-major with 8 live PSUM
    accumulators while x streams in; steady phase: remaining 12 n-tiles k-inner,
    ch-inner (one stationary per (nt,kt) feeds both 512-token chunks).
  - PSUM evicted via one DVE tensor_scalar (x*scale + bias, per-partition
    scalars); outputs stored as out.T f32 on the SP ring.
  - host: concat core outputs along n, transpose to [1024, 16384].
"""

import numpy as np

import concourse.tile as tile
from concourse import bacc, mybir
from concourse.bass_utils import run_bass_kernel_spmd

B, S, K, N = 2, 512, 4096, 16384
M = B * S              # 1024 tokens
NCORES = 8
NSH = N // NCORES      # 2048 out-features per core
P = 128
KT = K // P            # 32 k-tiles
NT = NSH // P          # 16 n-tiles per core
MCW = 512              # moving free dim per matmul (= one PSUM bank of f32)
MCH = M // MCW         # 2 token chunks
NT_A = 4               # n-tiles processed in the startup phase
WSPL = 8               # startup W tiles split at this k-tile
NDUMMY = 7             # warm-up matmuls

# x pieces beyond kt0/kt1: 2 k-tiles each
XG2 = (KT - 2) // 2


def build(w_bufs: int = 4, x_bufs: int = 3, psum_bufs: int = 8):
    nc = bacc.Bacc("TRN2", target_bir_lowering=False, debug=False)
    xT_d = nc.dram_tensor("xT", [K, M], mybir.dt.float32, kind="ExternalInput")
    w_d = nc.dram_tensor("wq", [NT * P, KT, P], mybir.dt.int8, kind="ExternalInput")
    s_d = nc.dram_tensor("scale", [P, NT], mybir.dt.float32, kind="ExternalInput")
    b_d = nc.dram_tensor("bias", [P, NT], mybir.dt.float32, kind="ExternalInput")
    o_d = nc.dram_tensor("outT", [NSH, M], mybir.dt.float32, kind="ExternalOutput")

    with tile.TileContext(nc) as tc:
        with (
            tc.tile_pool(name="xT_pool", bufs=1) as xT_pool,
            tc.tile_pool(name="xstage", bufs=x_bufs) as xstage_pool,
            tc.tile_pool(name="x0stage", bufs=1) as x0stage_pool,
            tc.tile_pool(name="w8", bufs=w_bufs) as w8_pool,
            tc.tile_pool(name="w8ab", bufs=1) as w8ab_pool,
            tc.tile_pool(name="wT_pool", bufs=w_bufs) as wT_pool,
            tc.tile_pool(name="wTab", bufs=1) as wTab_pool,
            tc.tile_pool(name="small", bufs=2) as small_pool,
            tc.tile_pool(name="osb", bufs=4) as osb_pool,
            tc.tile_pool(name="psum", bufs=psum_bufs, space="PSUM") as psum_pool,
        ):
            # ---- PE warm-up: dummy matmuls on a zeroed tile during DMA dead time
            dummy = small_pool.tile([P, MCW], mybir.dt.bfloat16, tag="dummy")
            nc.vector.memset(dummy[:], 0.0)

            psA = [
                [
                    psum_pool.tile(
                        [P, MCW], mybir.dt.float32, name=f"psA{nt}_{c}", tag="ps"
                    )
                    for c in range(MCH)
                ]
                for nt in range(NT_A)
            ]
            for i in range(NDUMMY):
                nc.tensor.matmul(
                    psA[0][0][:], dummy[:, 0:P], dummy[:], start=True, stop=True
                )

            # ---- startup DMAs.
            # SP ring: W kt0..WSPL pieces for the first NT_A n-tiles, then the
            # remainders.  ACT ring: x kt0 halves, kt1, then 2-kt pieces.
            w8a, w8b = {}, {}
            for nt in range(NT_A):
                w8a[nt] = w8ab_pool.tile(
                    [P, WSPL, P], mybir.dt.int8, name=f"w8a{nt}", tag=f"w8a{nt}"
                )
                nc.sync.dma_start(
                    out=w8a[nt][:], in_=w_d.ap()[nt * P:(nt + 1) * P, 0:WSPL]
                )
            for nt in range(NT_A):
                w8b[nt] = w8ab_pool.tile(
                    [P, KT - WSPL, P], mybir.dt.int8, name=f"w8b{nt}",
                    tag=f"w8b{nt}"
                )
                nc.sync.dma_start(
                    out=w8b[nt][:], in_=w_d.ap()[nt * P:(nt + 1) * P, WSPL:KT]
                )

            x0stg = []
            for h in range(MCH):
                stg = x0stage_pool.tile(
                    [P, 1, MCW], mybir.dt.float32, name=f"x0stg{h}", tag=f"x0stg{h}"
                )
                nc.scalar.dma_start(
                    out=stg[:],
                    in_=xT_d.ap()[0:P, h * MCW:(h + 1) * MCW].rearrange(
                        "(kt p) m -> p kt m", p=P
                    ),
                )
                x0stg.append(stg)
            x1stg = x0stage_pool.tile(
                [P, 1, M], mybir.dt.float32, name="x1stg", tag="x1stg"
            )
            nc.scalar.dma_start(
                out=x1stg[:],
                in_=xT_d.ap()[P:2 * P, :].rearrange("(kt p) m -> p kt m", p=P),
            )
            xstgs = [None] * XG2   # group g covers kt 2+2g, 3+2g
            for g in range(XG2):
                k0 = 2 + 2 * g
                xstg = xstage_pool.tile(
                    [P, 2, M], mybir.dt.float32, name=f"xstg{g}", tag="xstg"
                )
                nc.scalar.dma_start(
                    out=xstg[:],
                    in_=xT_d.ap()[k0 * P:(k0 + 2) * P, :].rearrange(
                        "(kt p) m -> p kt m", p=P
                    ),
                )
                xstgs[g] = xstg
            s_sb = small_pool.tile([P, NT], mybir.dt.float32, tag="s_sb")
            nc.sync.dma_start(out=s_sb[:], in_=s_d.ap())
            b_sb = small_pool.tile([P, NT], mybir.dt.float32, tag="b_sb")
            nc.sync.dma_start(out=b_sb[:], in_=b_d.ap())

            # ---- DVE cast chain (in-order!): wa pieces + earliest x first,
            # W-b pieces and phase-B prefetch only where PE slack allows.
            wTa, wTb = {}, {}
            xT0 = [None] * MCH
            xT1 = None
            xTg = [None] * XG2

            def cast_wa(nt):
                wTa[nt] = wTab_pool.tile(
                    [P, WSPL, P], mybir.dt.bfloat16, name=f"wTa{nt}", tag=f"wTa{nt}"
                )
                nc.vector.tensor_copy(out=wTa[nt][:], in_=w8a[nt][:])

            def cast_wb(nt):
                wTb[nt] = wTab_pool.tile(
                    [P, KT - WSPL, P], mybir.dt.bfloat16, name=f"wTb{nt}",
                    tag=f"wTb{nt}"
                )
                nc.vector.tensor_copy(out=wTb[nt][:], in_=w8b[nt][:])

            def cast_xg(g):
                xt = xT_pool.tile(
                    [P, 2, M], mybir.dt.bfloat16, name=f"xT{g}", tag=f"xTg{g}"
                )
                nc.vector.tensor_copy(out=xt[:], in_=xstgs[g][:])
                xTg[g] = xt

            cast_wa(0)
            xT0[0] = xT_pool.tile([P, 1, MCW], mybir.dt.bfloat16, name="xT00",
                                  tag="xT00")
            nc.vector.tensor_copy(out=xT0[0][:], in_=x0stg[0][:])
            cast_wa(1)
            cast_wa(2)
            xT0[1] = xT_pool.tile([P, 1, MCW], mybir.dt.bfloat16, name="xT01",
                                  tag="xT01")
            nc.vector.tensor_copy(out=xT0[1][:], in_=x0stg[1][:])
            cast_wa(3)
            xT1 = xT_pool.tile([P, 1, M], mybir.dt.bfloat16, name="xT1", tag="xT1")
            nc.vector.tensor_copy(out=xT1[:], in_=x1stg[:])
            cast_xg(0)   # kt2-3
            cast_xg(1)   # kt4-5
            cast_wb(0)
            cast_xg(2)   # kt6-7
            cast_wb(1)
            cast_xg(3)   # kt8-9
            cast_wb(2)
            cast_xg(4)   # kt10-11
            cast_wb(3)
            for g in range(5, XG2):
                cast_xg(g)

            # phase-B prefetch: DMA + DVE cast, after the x chain.
            wTs = {}

            def load_w_full(nt):
                w8 = w8_pool.tile([P, KT, P], mybir.dt.int8, name=f"w8_{nt}", tag="w8")
                nc.sync.dma_start(out=w8[:], in_=w_d.ap()[nt * P:(nt + 1) * P])
                wT = wT_pool.tile(
                    [P, KT, P], mybir.dt.bfloat16, name=f"wT{nt}", tag="wT"
                )
                nc.vector.tensor_copy(out=wT[:], in_=w8[:])
                wTs[nt] = wT

            for nt in range(NT_A, min(NT_A + 4, NT)):
                load_w_full(nt)

            def stationary(nt, kt):
                if nt < NT_A:
                    if kt < WSPL:
                        return wTa[nt][:, kt, :]
                    return wTb[nt][:, kt - WSPL, :]
                return wTs[nt][:, kt, :]

            def moving(kt, c):
                if kt == 0:
                    return xT0[c][:, 0, :]
                if kt == 1:
                    return xT1[:, 0, c * MCW:(c + 1) * MCW]
                g, kti = divmod(kt - 2, 2)
                return xTg[g][:, kti, c * MCW:(c + 1) * MCW]

            # ---- phase A matmuls: k-major across NT_A n-tiles.
            # kt0 runs c-outer so the second m-half's cast can trail the first.
            for c in range(MCH):
                for nt in range(NT_A):
                    nc.tensor.matmul(
                        psA[nt][c][:], stationary(nt, 0), moving(0, c),
                        start=True, stop=False,
                    )
            for kt in range(1, KT):
                for nt in range(NT_A):
                    for c in range(MCH):
                        nc.tensor.matmul(
                            psA[nt][c][:], stationary(nt, kt), moving(kt, c),
                            start=False, stop=(kt == KT - 1),
                        )

            def evict(nt, c, ps):
                o_sb = osb_pool.tile(
                    [P, MCW], mybir.dt.float32, name=f"osb{nt}_{c}", tag="o_sb"
                )
                nc.vector.tensor_scalar(
                    out=o_sb[:],
                    in0=ps[:],
                    scalar1=s_sb[:, nt:nt + 1],
                    scalar2=b_sb[:, nt:nt + 1],
                    op0=mybir.AluOpType.mult,
                    op1=mybir.AluOpType.add,
                )
                nc.sync.dma_start(
                    out=o_d.ap()[nt * P:(nt + 1) * P, c * MCW:(c + 1) * MCW],
                    in_=o_sb[:],
                )

            for nt in range(NT_A):
                for c in range(MCH):
                    evict(nt, c, psA[nt][c])

            # ---- phase B: remaining n-tiles, k-inner / ch-inner
            for nt in range(NT_A, NT):
                ps = [
                    psum_pool.tile(
                        [P, MCW], mybir.dt.float32, name=f"ps{nt}_{c}", tag="ps"
                    )
                    for c in range(MCH)
                ]
                for kt in range(KT):
                    for c in range(MCH):
                        nc.tensor.matmul(
                            ps[c][:], stationary(nt, kt), moving(kt, c),
                            start=(kt == 0), stop=(kt == KT - 1),
                        )
                if nt + 4 < NT:
                    load_w_full(nt + 4)
                for c in range(MCH):
                    evict(nt, c, ps[c])

    nc.compile()
    return nc


def make_in_maps(x, weight_quant, scale, bias):
    x2T = np.ascontiguousarray(
        np.asarray(x, dtype=np.float32).reshape(M, K).T
    )  # [K, M] k-major replica
    wq = np.asarray(weight_quant, dtype=np.int32).astype(np.int8)  # int8-valued
    scale = np.asarray(scale, dtype=np.float32)
    bias = np.asarray(bias, dtype=np.float32)
    in_maps = []
    for i in range(NCORES):
        sl = slice(i * NSH, (i + 1) * NSH)
        # [nsh, k] -> [nt, n, kt, p] -> [nt, p, kt, n] -> [nt*p, kt, n]
        w_sw = np.ascontiguousarray(
            wq[sl].reshape(NT, P, KT, P).transpose(0, 3, 2, 1)
        ).reshape(NT * P, KT, P)
        in_maps.append({
            "xT": x2T,
            "wq": w_sw,
            "scale": np.ascontiguousarray(scale[sl].reshape(NT, P).T),
            "bias": np.ascontiguousarray(bias[sl].reshape(NT, P).T),
        })
    return in_maps


def gather_output(results):
    outT = np.concatenate([np.asarray(r["outT"]) for r in results], axis=0)  # [N, M]
    return np.ascontiguousarray(outT.T).reshape(B, S, N).astype(np.float32, copy=False)


def kernel(x, weight_quant, scale, bias):
    nc = build()
    in_maps = make_in_maps(x, weight_quant, scale, bias)
    res = run_bass_kernel_spmd(nc, in_maps, core_ids=list(range(NCORES)))
    return gather_output(res.results)


if __name__ == "__main__":
    rng = np.random.default_rng(0)
    x = rng.standard_normal((B, S, K), dtype=np.float32)
    wq = rng.integers(-128, 128, size=(N, K), dtype=np.int64).astype(np.int32)
    scale = rng.uniform(0.001, 0.02, size=(N,)).astype(np.float32)
    bias = rng.standard_normal((N,), dtype=np.float32)
    out = kernel(x=x, weight_quant=wq, scale=scale, bias=bias)
    w = wq.astype(np.float32) * scale[:, None]
    exp = x.reshape(M, K) @ w.T + bias
    err = np.abs(out.reshape(M, N) - exp).max() / np.abs(exp).max()
    print("self-check rel err:", err)


# revision 15
# speedup vs baseline: 1.3111x; 1.0436x over previous
"""nn_Linear8bit on 8 TRN2 NeuronCores — column-parallel (tensor-parallel on out_features).

out[m, n] = sum_k x[m, k] * wq[n, k] * scale[n] + bias[n]
  x: [2, 512, 4096] f32, wq: [16384, 4096] int32 (int8-valued), scale/bias: [16384] f32

Sharding: W/scale/bias row-sharded 2048/core; x replicated (fed k-major). No collectives.

Host prep (pure layout/bit repack, no arithmetic):
  - x -> x.T [K, M] f32 (k-major replica).
  - wq (int8-valued int32) -> int8, transposed+swizzled to [nt*128, kt, n] so each
    n-tile's stationary block DMAs as contiguous partition lines.
  - scale/bias -> [128, 16] (partition-major per n-tile).

Per-core dataflow (all HWDGE, all casts on DVE, no on-chip transposes):
  - x: f32 DMA on the ACT HWDGE ring — kt0 in two 512-token halves, kt1 alone,
    then 2-k-tile pieces — so the first matmul starts ~11.5us in and arrivals
    stay smooth.  DVE casts f32->bf16 into resident xT tiles.
  - W: int8 DMA on the SP HWDGE ring; first 4 n-tiles split (kt 0..7 / 8..31)
    into separate tiles.  All W casts int8->bf16 run on DVE, interleaved into
    the x-cast chain at points where the PE won't need the result yet (the
    in-order DVE queue must never delay an x cast past its matmul).  GpSimd is
    deliberately unused: its cast ucode is ~25x slower and stalls concurrent
    DVE ops via SBUF port contention (measured).
  - 7 dummy warm-up matmuls on a memset tile run during the initial DMA dead
    time so the PE_HAM clock-gate is at 8/8 when real matmuls start.
  - Startup phase: first 4 n-tiles processed k-major with 8 live PSUM
    accumulators while x streams in; steady phase: remaining 12 n-tiles k-inner,
    ch-inner (one stationary per (nt,kt) feeds both 512-token chunks).
  - PSUM evicted via one DVE tensor_scalar (x*scale + bias, per-partition
    scalars); outputs stored as out.T f32 on the SP ring.
  - host: concat core outputs along n, transpose to [1024, 16384].
"""

import numpy as np

import concourse.tile as tile
from concourse import bacc, mybir
from concourse.bass_utils import run_bass_kernel_spmd

B, S, K, N = 2, 512, 4096, 16384
M = B * S              # 1024 tokens
NCORES = 8
NSH = N // NCORES      # 2048 out-features per core
P = 128
KT = K // P            # 32 k-tiles
NT = NSH // P          # 16 n-tiles per core
MCW = 512              # moving free dim per matmul (= one PSUM bank of f32)
MCH = M // MCW         # 2 token chunks
NT_A = 4               # n-tiles processed in the startup phase
WSPL = 8               # startup W tiles split at this k-tile
NDUMMY = 7             # warm-up matmuls

# x pieces beyond kt0/kt1: 2 k-tiles each
XG2 = (KT - 2) // 2


def build(w_bufs: int = 4, x_bufs: int = 3, psum_bufs: int = 8):
    nc = bacc.Bacc("TRN2", target_bir_lowering=False, debug=False)
    xT_d = nc.dram_tensor("xT", [K, M], mybir.dt.float32, kind="ExternalInput")
    w_d = nc.dram_tensor("wq", [NT * P, KT, P], mybir.dt.int8, kind="ExternalInput")
    s_d = nc.dram_tensor("scale", [P, NT], mybir.dt.float32, kind="ExternalInput")
    b_d = nc.dram_tensor("bias", [P, NT], mybir.dt.float32, kind="ExternalInput")
    o_d = nc.dram_tensor("outT", [NSH, M], mybir.dt.float32, kind="ExternalOutput")

    with tile.TileContext(nc) as tc:
        with (
            tc.tile_pool(name="xT_pool", bufs=1) as xT_pool,
            tc.tile_pool(name="xstage", bufs=x_bufs) as xstage_pool,
            tc.tile_pool(name="x0stage", bufs=1) as x0stage_pool,
            tc.tile_pool(name="w8", bufs=w_bufs) as w8_pool,
            tc.tile_pool(name="w8ab", bufs=1) as w8ab_pool,
            tc.tile_pool(name="wT_pool", bufs=w_bufs) as wT_pool,
            tc.tile_pool(name="wTab", bufs=1) as wTab_pool,
            tc.tile_pool(name="small", bufs=2) as small_pool,
            tc.tile_pool(name="osb", bufs=4) as osb_pool,
            tc.tile_pool(name="psum", bufs=psum_bufs, space="PSUM") as psum_pool,
        ):
            # ---- PE warm-up: dummy matmuls on a zeroed tile during DMA dead time
            dummy = small_pool.tile([P, MCW], mybir.dt.bfloat16, tag="dummy")
            nc.vector.memset(dummy[:], 0.0)

            psA = [
                [
                    psum_pool.tile(
                        [P, MCW], mybir.dt.float32, name=f"psA{nt}_{c}", tag="ps"
                    )
                    for c in range(MCH)
                ]
                for nt in range(NT_A)
            ]
            for i in range(NDUMMY):
                nc.tensor.matmul(
                    psA[0][0][:], dummy[:, 0:P], dummy[:], start=True, stop=True
                )

            # ---- startup DMAs.
            # SP ring: W kt0..WSPL pieces for the first NT_A n-tiles, then the
            # remainders.  ACT ring: x kt0 halves, kt1, then 2-kt pieces.
            w8a, w8b = {}, {}
            for nt in range(NT_A):
                w8a[nt] = w8ab_pool.tile(
                    [P, WSPL, P], mybir.dt.int8, name=f"w8a{nt}", tag=f"w8a{nt}"
                )
                nc.sync.dma_start(
                    out=w8a[nt][:], in_=w_d.ap()[nt * P:(nt + 1) * P, 0:WSPL]
                )
            for nt in range(NT_A):
                w8b[nt] = w8ab_pool.tile(
                    [P, KT - WSPL, P], mybir.dt.int8, name=f"w8b{nt}",
                    tag=f"w8b{nt}"
                )
                nc.sync.dma_start(
                    out=w8b[nt][:], in_=w_d.ap()[nt * P:(nt + 1) * P, WSPL:KT]
                )

            x0stg = []
            for h in range(MCH):
                stg = x0stage_pool.tile(
                    [P, 1, MCW], mybir.dt.float32, name=f"x0stg{h}", tag=f"x0stg{h}"
                )
                nc.scalar.dma_start(
                    out=stg[:],
                    in_=xT_d.ap()[0:P, h * MCW:(h + 1) * MCW].rearrange(
                        "(kt p) m -> p kt m", p=P
                    ),
                )
                x0stg.append(stg)
            x1stg = x0stage_pool.tile(
                [P, 1, M], mybir.dt.float32, name="x1stg", tag="x1stg"
            )
            nc.scalar.dma_start(
                out=x1stg[:],
                in_=xT_d.ap()[P:2 * P, :].rearrange("(kt p) m -> p kt m", p=P),
            )
            xstgs = [None] * XG2   # group g covers kt 2+2g, 3+2g
            for g in range(XG2):
                k0 = 2 + 2 * g
                xstg = xstage_pool.tile(
                    [P, 2, M], mybir.dt.float32, name=f"xstg{g}", tag="xstg"
                )
                nc.scalar.dma_start(
                    out=xstg[:],
                    in_=xT_d.ap()[k0 * P:(k0 + 2) * P, :].rearrange(
                        "(kt p) m -> p kt m", p=P
                    ),
                )
                xstgs[g] = xstg
            # ---- DVE cast chain (in-order!): wa pieces + earliest x first,
            # W-b pieces and phase-B prefetch only where PE slack allows.
            wTa, wTb = {}, {}
            xT0 = [None] * MCH
            xT1 = None
            xTg = [None] * XG2

            def cast_wa(nt):
                wTa[nt] = wTab_pool.tile(
                    [P, WSPL, P], mybir.dt.bfloat16, name=f"wTa{nt}", tag=f"wTa{nt}"
                )
                nc.vector.tensor_copy(out=wTa[nt][:], in_=w8a[nt][:])

            def cast_wb(nt):
                wTb[nt] = wTab_pool.tile(
                    [P, KT - WSPL, P], mybir.dt.bfloat16, name=f"wTb{nt}",
                    tag=f"wTb{nt}"
                )
                nc.vector.tensor_copy(out=wTb[nt][:], in_=w8b[nt][:])

            def cast_xg(g):
                xt = xT_pool.tile(
                    [P, 2, M], mybir.dt.bfloat16, name=f"xT{g}", tag=f"xTg{g}"
                )
                nc.vector.tensor_copy(out=xt[:], in_=xstgs[g][:])
                xTg[g] = xt

            cast_wa(0)
            xT0[0] = xT_pool.tile([P, 1, MCW], mybir.dt.bfloat16, name="xT00",
                                  tag="xT00")
            nc.vector.tensor_copy(out=xT0[0][:], in_=x0stg[0][:])
            cast_wa(1)
            cast_wa(2)
            xT0[1] = xT_pool.tile([P, 1, MCW], mybir.dt.bfloat16, name="xT01",
                                  tag="xT01")
            nc.vector.tensor_copy(out=xT0[1][:], in_=x0stg[1][:])
            cast_wa(3)
            xT1 = xT_pool.tile([P, 1, M], mybir.dt.bfloat16, name="xT1", tag="xT1")
            nc.vector.tensor_copy(out=xT1[:], in_=x1stg[:])
            cast_xg(0)   # kt2-3
            cast_xg(1)   # kt4-5
            cast_wb(0)
            cast_xg(2)   # kt6-7
            cast_wb(1)
            cast_xg(3)   # kt8-9
            cast_wb(2)
            cast_xg(4)   # kt10-11
            cast_wb(3)
            for g in range(5, XG2):
                cast_xg(g)

            wTs = {}

            def load_w_full(nt):
                # ACT ring: FIFO-ordered behind the x stream, so these 512KB
                # loads can't steal HBM bandwidth from phase A's x feed.
                w8 = w8_pool.tile([P, KT, P], mybir.dt.int8, name=f"w8_{nt}", tag="w8")
                nc.scalar.dma_start(out=w8[:], in_=w_d.ap()[nt * P:(nt + 1) * P])
                wT = wT_pool.tile(
                    [P, KT, P], mybir.dt.bfloat16, name=f"wT{nt}", tag="wT"
                )
                nc.vector.tensor_copy(out=wT[:], in_=w8[:])
                wTs[nt] = wT

            def stationary(nt, kt):
                if nt < NT_A:
                    if kt < WSPL:
                        return wTa[nt][:, kt, :]
                    return wTb[nt][:, kt - WSPL, :]
                return wTs[nt][:, kt, :]

            def moving(kt, c):
                if kt == 0:
                    return xT0[c][:, 0, :]
                if kt == 1:
                    return xT1[:, 0, c * MCW:(c + 1) * MCW]
                g, kti = divmod(kt - 2, 2)
                return xTg[g][:, kti, c * MCW:(c + 1) * MCW]

            # ---- phase A matmuls: k-major across NT_A n-tiles.
            # kt0 runs c-outer so the second m-half's cast can trail the first.
            for c in range(MCH):
                for nt in range(NT_A):
                    nc.tensor.matmul(
                        psA[nt][c][:], stationary(nt, 0), moving(0, c),
                        start=True, stop=False,
                    )
            for kt in range(1, KT):
                for nt in range(NT_A):
                    for c in range(MCH):
                        nc.tensor.matmul(
                            psA[nt][c][:], stationary(nt, kt), moving(kt, c),
                            start=False, stop=(kt == KT - 1),
                        )

            # phase-B prefetch + scale/bias: DMAs deferred to here so their
            # HBM traffic stays out of the x-stream window phase A feeds from.
            s_sb = small_pool.tile([P, NT], mybir.dt.float32, tag="s_sb")
            nc.scalar.dma_start(out=s_sb[:], in_=s_d.ap())
            b_sb = small_pool.tile([P, NT], mybir.dt.float32, tag="b_sb")
            nc.scalar.dma_start(out=b_sb[:], in_=b_d.ap())
            for nt in range(NT_A, min(NT_A + 4, NT)):
                load_w_full(nt)

            def evict(nt, c, ps):
                o_sb = osb_pool.tile(
                    [P, MCW], mybir.dt.float32, name=f"osb{nt}_{c}", tag="o_sb"
                )
                nc.vector.tensor_scalar(
                    out=o_sb[:],
                    in0=ps[:],
                    scalar1=s_sb[:, nt:nt + 1],
                    scalar2=b_sb[:, nt:nt + 1],
                    op0=mybir.AluOpType.mult,
                    op1=mybir.AluOpType.add,
                )
                nc.sync.dma_start(
                    out=o_d.ap()[nt * P:(nt + 1) * P, c * MCW:(c + 1) * MCW],
                    in_=o_sb[:],
                )

            for nt in range(NT_A):
                for c in range(MCH):
                    evict(nt, c, psA[nt][c])

            # ---- phase B: remaining n-tiles, k-inner / ch-inner
            for nt in range(NT_A, NT):
                ps = [
                    psum_pool.tile(
                        [P, MCW], mybir.dt.float32, name=f"ps{nt}_{c}", tag="ps"
                    )
                    for c in range(MCH)
                ]
                for kt in range(KT):
                    for c in range(MCH):
                        nc.tensor.matmul(
                            ps[c][:], stationary(nt, kt), moving(kt, c),
                            start=(kt == 0), stop=(kt == KT - 1),
                        )
                if nt + 4 < NT:
                    load_w_full(nt + 4)
                for c in range(MCH):
                    evict(nt, c, ps[c])

    nc.compile()
    return nc


def make_in_maps(x, weight_quant, scale, bias):
    x2T = np.ascontiguousarray(
        np.asarray(x, dtype=np.float32).reshape(M, K).T
    )  # [K, M] k-major replica
    wq = np.asarray(weight_quant, dtype=np.int32).astype(np.int8)  # int8-valued
    scale = np.asarray(scale, dtype=np.float32)
    bias = np.asarray(bias, dtype=np.float32)
    in_maps = []
    for i in range(NCORES):
        sl = slice(i * NSH, (i + 1) * NSH)
        # [nsh, k] -> [nt, n, kt, p] -> [nt, p, kt, n] -> [nt*p, kt, n]
        w_sw = np.ascontiguousarray(
            wq[sl].reshape(NT, P, KT, P).transpose(0, 3, 2, 1)
        ).reshape(NT * P, KT, P)
        in_maps.append({
            "xT": x2T,
            "wq": w_sw,
            "scale": np.ascontiguousarray(scale[sl].reshape(NT, P).T),
            "bias": np.ascontiguousarray(bias[sl].reshape(NT, P).T),
        })
    return in_maps


def gather_output(results):
    outT = np.concatenate([np.asarray(r["outT"]) for r in results], axis=0)  # [N, M]
    return np.ascontiguousarray(outT.T).reshape(B, S, N).astype(np.float32, copy=False)


def kernel(x, weight_quant, scale, bias):
    nc = build()
    in_maps = make_in_maps(x, weight_quant, scale, bias)
    res = run_bass_kernel_spmd(nc, in_maps, core_ids=list(range(NCORES)))
    return gather_output(res.results)


if __name__ == "__main__":
    rng = np.random.default_rng(0)
    x = rng.standard_normal((B, S, K), dtype=np.float32)
    wq = rng.integers(-128, 128, size=(N, K), dtype=np.int64).astype(np.int32)
    scale = rng.uniform(0.001, 0.02, size=(N,)).astype(np.float32)
    bias = rng.standard_normal((N,), dtype=np.float32)
    out = kernel(x=x, weight_quant=wq, scale=scale, bias=bias)
    w = wq.astype(np.float32) * scale[:, None]
    exp = x.reshape(M, K) @ w.T + bias
    err = np.abs(out.reshape(M, N) - exp).max() / np.abs(exp).max()
    print("self-check rel err:", err)


# revision 16
# speedup vs baseline: 1.3153x; 1.0032x over previous
"""nn_Linear8bit on 8 TRN2 NeuronCores — column-parallel (tensor-parallel on out_features).

out[m, n] = sum_k x[m, k] * wq[n, k] * scale[n] + bias[n]
  x: [2, 512, 4096] f32, wq: [16384, 4096] int32 (int8-valued), scale/bias: [16384] f32

Sharding: W/scale/bias row-sharded 2048/core; x replicated (fed k-major). No collectives.

Host prep (pure layout/bit repack, no arithmetic):
  - x -> x.T [K, M] f32 (k-major replica).
  - wq (int8-valued int32) -> int8, transposed+swizzled to [nt*128, kt, n] so each
    n-tile's stationary block DMAs as contiguous partition lines.
  - scale/bias -> [128, 16] (partition-major per n-tile).

Per-core dataflow (all HWDGE, all casts on DVE, no on-chip transposes):
  - x: f32 DMA on the ACT HWDGE ring — kt0 in two 512-token halves, kt1 alone,
    then 2-k-tile pieces — so the first matmul starts ~11.5us in and arrivals
    stay smooth.  DVE casts f32->bf16 into resident xT tiles.
  - W: int8 DMA on the SP HWDGE ring; first 4 n-tiles split (kt 0..7 / 8..31)
    into separate tiles.  All W casts int8->bf16 run on DVE, interleaved into
    the x-cast chain at points where the PE won't need the result yet (the
    in-order DVE queue must never delay an x cast past its matmul).  GpSimd is
    deliberately unused: its cast ucode is ~25x slower and stalls concurrent
    DVE ops via SBUF port contention (measured).
  - 7 dummy warm-up matmuls on a memset tile run during the initial DMA dead
    time so the PE_HAM clock-gate is at 8/8 when real matmuls start.
  - Startup phase: first 4 n-tiles processed k-major with 8 live PSUM
    accumulators while x streams in; steady phase: remaining 12 n-tiles k-inner,
    ch-inner (one stationary per (nt,kt) feeds both 512-token chunks).
  - PSUM evicted via one DVE tensor_scalar (x*scale + bias, per-partition
    scalars); outputs stored as out.T f32 on the SP ring.
  - host: concat core outputs along n, transpose to [1024, 16384].
"""

import numpy as np

import concourse.tile as tile
from concourse import bacc, mybir
from concourse.bass_utils import run_bass_kernel_spmd

B, S, K, N = 2, 512, 4096, 16384
M = B * S              # 1024 tokens
NCORES = 8
NSH = N // NCORES      # 2048 out-features per core
P = 128
KT = K // P            # 32 k-tiles
NT = NSH // P          # 16 n-tiles per core
MCW = 512              # moving free dim per matmul (= one PSUM bank of f32)
MCH = M // MCW         # 2 token chunks
NT_A = 4               # n-tiles processed in the startup phase
WSPL = 8               # startup W tiles split at this k-tile
NDUMMY = 16            # warm-up matmuls (cover the DMA/cast dead time ~8-14us)

# x pieces beyond kt0/kt1: 2 k-tiles each
XG2 = (KT - 2) // 2


def build(w_bufs: int = 4, x_bufs: int = 3, psum_bufs: int = 8):
    nc = bacc.Bacc("TRN2", target_bir_lowering=False, debug=False)
    xT_d = nc.dram_tensor("xT", [K, M], mybir.dt.float32, kind="ExternalInput")
    w_d = nc.dram_tensor("wq", [NT * P, KT, P], mybir.dt.int8, kind="ExternalInput")
    s_d = nc.dram_tensor("scale", [P, NT], mybir.dt.float32, kind="ExternalInput")
    b_d = nc.dram_tensor("bias", [P, NT], mybir.dt.float32, kind="ExternalInput")
    o_d = nc.dram_tensor("outT", [NSH, M], mybir.dt.float32, kind="ExternalOutput")

    with tile.TileContext(nc) as tc:
        with (
            tc.tile_pool(name="xT_pool", bufs=1) as xT_pool,
            tc.tile_pool(name="xstage", bufs=x_bufs) as xstage_pool,
            tc.tile_pool(name="x0stage", bufs=1) as x0stage_pool,
            tc.tile_pool(name="w8", bufs=w_bufs) as w8_pool,
            tc.tile_pool(name="w8ab", bufs=1) as w8ab_pool,
            tc.tile_pool(name="wT_pool", bufs=w_bufs) as wT_pool,
            tc.tile_pool(name="wTab", bufs=1) as wTab_pool,
            tc.tile_pool(name="small", bufs=2) as small_pool,
            tc.tile_pool(name="osb", bufs=4) as osb_pool,
            tc.tile_pool(name="psum", bufs=psum_bufs, space="PSUM") as psum_pool,
        ):
            # ---- PE warm-up: dummy matmuls on a zeroed tile during DMA dead time
            dummy = small_pool.tile([P, MCW], mybir.dt.bfloat16, tag="dummy")
            nc.vector.memset(dummy[:], 0.0)

            psA = [
                [
                    psum_pool.tile(
                        [P, MCW], mybir.dt.float32, name=f"psA{nt}_{c}", tag="ps"
                    )
                    for c in range(MCH)
                ]
                for nt in range(NT_A)
            ]
            for i in range(NDUMMY):
                nc.tensor.matmul(
                    psA[0][0][:], dummy[:, 0:P], dummy[:], start=True, stop=True
                )

            # ---- startup DMAs.
            # SP ring: W kt0..WSPL pieces for the first NT_A n-tiles, then the
            # remainders.  ACT ring: x kt0 halves, kt1, then 2-kt pieces.
            w8a, w8b = {}, {}
            for nt in range(NT_A):
                w8a[nt] = w8ab_pool.tile(
                    [P, WSPL, P], mybir.dt.int8, name=f"w8a{nt}", tag=f"w8a{nt}"
                )
                nc.sync.dma_start(
                    out=w8a[nt][:], in_=w_d.ap()[nt * P:(nt + 1) * P, 0:WSPL]
                )
            for nt in range(NT_A):
                w8b[nt] = w8ab_pool.tile(
                    [P, KT - WSPL, P], mybir.dt.int8, name=f"w8b{nt}",
                    tag=f"w8b{nt}"
                )
                nc.sync.dma_start(
                    out=w8b[nt][:], in_=w_d.ap()[nt * P:(nt + 1) * P, WSPL:KT]
                )

            x0stg = []
            for h in range(MCH):
                stg = x0stage_pool.tile(
                    [P, 1, MCW], mybir.dt.float32, name=f"x0stg{h}", tag=f"x0stg{h}"
                )
                nc.scalar.dma_start(
                    out=stg[:],
                    in_=xT_d.ap()[0:P, h * MCW:(h + 1) * MCW].rearrange(
                        "(kt p) m -> p kt m", p=P
                    ),
                )
                x0stg.append(stg)
            x1stg = x0stage_pool.tile(
                [P, 1, M], mybir.dt.float32, name="x1stg", tag="x1stg"
            )
            nc.scalar.dma_start(
                out=x1stg[:],
                in_=xT_d.ap()[P:2 * P, :].rearrange("(kt p) m -> p kt m", p=P),
            )
            xstgs = [None] * XG2   # group g covers kt 2+2g, 3+2g
            for g in range(XG2):
                k0 = 2 + 2 * g
                xstg = xstage_pool.tile(
                    [P, 2, M], mybir.dt.float32, name=f"xstg{g}", tag="xstg"
                )
                nc.scalar.dma_start(
                    out=xstg[:],
                    in_=xT_d.ap()[k0 * P:(k0 + 2) * P, :].rearrange(
                        "(kt p) m -> p kt m", p=P
                    ),
                )
                xstgs[g] = xstg
            # ---- DVE cast chain (in-order!): wa pieces + earliest x first,
            # W-b pieces and phase-B prefetch only where PE slack allows.
            wTa, wTb = {}, {}
            xT0 = [None] * MCH
            xT1 = None
            xTg = [None] * XG2

            def cast_wa(nt):
                wTa[nt] = wTab_pool.tile(
                    [P, WSPL, P], mybir.dt.bfloat16, name=f"wTa{nt}", tag=f"wTa{nt}"
                )
                nc.vector.tensor_copy(out=wTa[nt][:], in_=w8a[nt][:])

            def cast_wb(nt):
                wTb[nt] = wTab_pool.tile(
                    [P, KT - WSPL, P], mybir.dt.bfloat16, name=f"wTb{nt}",
                    tag=f"wTb{nt}"
                )
                nc.vector.tensor_copy(out=wTb[nt][:], in_=w8b[nt][:])

            def cast_xg(g):
                xt = xT_pool.tile(
                    [P, 2, M], mybir.dt.bfloat16, name=f"xT{g}", tag=f"xTg{g}"
                )
                nc.vector.tensor_copy(out=xt[:], in_=xstgs[g][:])
                xTg[g] = xt

            cast_wa(0)
            xT0[0] = xT_pool.tile([P, 1, MCW], mybir.dt.bfloat16, name="xT00",
                                  tag="xT00")
            nc.vector.tensor_copy(out=xT0[0][:], in_=x0stg[0][:])
            cast_wa(1)
            cast_wa(2)
            xT0[1] = xT_pool.tile([P, 1, MCW], mybir.dt.bfloat16, name="xT01",
                                  tag="xT01")
            nc.vector.tensor_copy(out=xT0[1][:], in_=x0stg[1][:])
            cast_wa(3)
            xT1 = xT_pool.tile([P, 1, M], mybir.dt.bfloat16, name="xT1", tag="xT1")
            nc.vector.tensor_copy(out=xT1[:], in_=x1stg[:])
            cast_xg(0)   # kt2-3
            cast_xg(1)   # kt4-5
            cast_wb(0)
            cast_xg(2)   # kt6-7
            cast_wb(1)
            cast_xg(3)   # kt8-9
            cast_wb(2)
            cast_xg(4)   # kt10-11
            cast_wb(3)
            for g in range(5, XG2):
                cast_xg(g)

            wTs = {}

            def load_w_full(nt):
                # ACT ring: FIFO-ordered behind the x stream, so these 512KB
                # loads can't steal HBM bandwidth from phase A's x feed.
                w8 = w8_pool.tile([P, KT, P], mybir.dt.int8, name=f"w8_{nt}", tag="w8")
                nc.scalar.dma_start(out=w8[:], in_=w_d.ap()[nt * P:(nt + 1) * P])
                wT = wT_pool.tile(
                    [P, KT, P], mybir.dt.bfloat16, name=f"wT{nt}", tag="wT"
                )
                nc.vector.tensor_copy(out=wT[:], in_=w8[:])
                wTs[nt] = wT

            def stationary(nt, kt):
                if nt < NT_A:
                    if kt < WSPL:
                        return wTa[nt][:, kt, :]
                    return wTb[nt][:, kt - WSPL, :]
                return wTs[nt][:, kt, :]

            def moving(kt, c):
                if kt == 0:
                    return xT0[c][:, 0, :]
                if kt == 1:
                    return xT1[:, 0, c * MCW:(c + 1) * MCW]
                g, kti = divmod(kt - 2, 2)
                return xTg[g][:, kti, c * MCW:(c + 1) * MCW]

            # ---- phase A matmuls: k-major across NT_A n-tiles.
            # kt0 runs c-outer so the second m-half's cast can trail the first.
            for c in range(MCH):
                for nt in range(NT_A):
                    nc.tensor.matmul(
                        psA[nt][c][:], stationary(nt, 0), moving(0, c),
                        start=True, stop=False,
                    )
            for kt in range(1, KT):
                for nt in range(NT_A):
                    for c in range(MCH):
                        nc.tensor.matmul(
                            psA[nt][c][:], stationary(nt, kt), moving(kt, c),
                            start=False, stop=(kt == KT - 1),
                        )

            # phase-B prefetch + scale/bias: DMAs deferred to here so their
            # HBM traffic stays out of the x-stream window phase A feeds from.
            s_sb = small_pool.tile([P, NT], mybir.dt.float32, tag="s_sb")
            nc.scalar.dma_start(out=s_sb[:], in_=s_d.ap())
            b_sb = small_pool.tile([P, NT], mybir.dt.float32, tag="b_sb")
            nc.scalar.dma_start(out=b_sb[:], in_=b_d.ap())
            for nt in range(NT_A, min(NT_A + 4, NT)):
                load_w_full(nt)

            def evict(nt, c, ps):
                o_sb = osb_pool.tile(
                    [P, MCW], mybir.dt.float32, name=f"osb{nt}_{c}", tag="o_sb"
                )
                nc.vector.tensor_scalar(
                    out=o_sb[:],
                    in0=ps[:],
                    scalar1=s_sb[:, nt:nt + 1],
                    scalar2=b_sb[:, nt:nt + 1],
                    op0=mybir.AluOpType.mult,
                    op1=mybir.AluOpType.add,
                )
                nc.sync.dma_start(
                    out=o_d.ap()[nt * P:(nt + 1) * P, c * MCW:(c + 1) * MCW],
                    in_=o_sb[:],
                )

            for nt in range(NT_A):
                for c in range(MCH):
                    evict(nt, c, psA[nt][c])

            # ---- phase B: remaining n-tiles, k-inner / ch-inner
            for nt in range(NT_A, NT):
                ps = [
                    psum_pool.tile(
                        [P, MCW], mybir.dt.float32, name=f"ps{nt}_{c}", tag="ps"
                    )
                    for c in range(MCH)
                ]
                for kt in range(KT):
                    for c in range(MCH):
                        nc.tensor.matmul(
                            ps[c][:], stationary(nt, kt), moving(kt, c),
                            start=(kt == 0), stop=(kt == KT - 1),
                        )
                if nt + 4 < NT:
                    load_w_full(nt + 4)
                for c in range(MCH):
                    evict(nt, c, ps[c])

    nc.compile()
    return nc


def make_in_maps(x, weight_quant, scale, bias):
    x2T = np.ascontiguousarray(
        np.asarray(x, dtype=np.float32).reshape(M, K).T
    )  # [K, M] k-major replica
    wq = np.asarray(weight_quant, dtype=np.int32).astype(np.int8)  # int8-valued
    scale = np.asarray(scale, dtype=np.float32)
    bias = np.asarray(bias, dtype=np.float32)
    in_maps = []
    for i in range(NCORES):
        sl = slice(i * NSH, (i + 1) * NSH)
        # [nsh, k] -> [nt, n, kt, p] -> [nt, p, kt, n] -> [nt*p, kt, n]
        w_sw = np.ascontiguousarray(
            wq[sl].reshape(NT, P, KT, P).transpose(0, 3, 2, 1)
        ).reshape(NT * P, KT, P)
        in_maps.append({
            "xT": x2T,
            "wq": w_sw,
            "scale": np.ascontiguousarray(scale[sl].reshape(NT, P).T),
            "bias": np.ascontiguousarray(bias[sl].reshape(NT, P).T),
        })
    return in_maps


def gather_output(results):
    outT = np.concatenate([np.asarray(r["outT"]) for r in results], axis=0)  # [N, M]
    return np.ascontiguousarray(outT.T).reshape(B, S, N).astype(np.float32, copy=False)


def kernel(x, weight_quant, scale, bias):
    nc = build()
    in_maps = make_in_maps(x, weight_quant, scale, bias)
    res = run_bass_kernel_spmd(nc, in_maps, core_ids=list(range(NCORES)))
    return gather_output(res.results)


if __name__ == "__main__":
    rng = np.random.default_rng(0)
    x = rng.standard_normal((B, S, K), dtype=np.float32)
    wq = rng.integers(-128, 128, size=(N, K), dtype=np.int64).astype(np.int32)
    scale = rng.uniform(0.001, 0.02, size=(N,)).astype(np.float32)
    bias = rng.standard_normal((N,), dtype=np.float32)
    out = kernel(x=x, weight_quant=wq, scale=scale, bias=bias)
    w = wq.astype(np.float32) * scale[:, None]
    exp = x.reshape(M, K) @ w.T + bias
    err = np.abs(out.reshape(M, N) - exp).max() / np.abs(exp).max()
    print("self-check rel err:", err)


# revision 17
# speedup vs baseline: 1.3190x; 1.0028x over previous
"""nn_Linear8bit on 8 TRN2 NeuronCores — column-parallel (tensor-parallel on out_features).

out[m, n] = sum_k x[m, k] * wq[n, k] * scale[n] + bias[n]
  x: [2, 512, 4096] f32, wq: [16384, 4096] int32 (int8-valued), scale/bias: [16384] f32

Sharding: W/scale/bias row-sharded 2048/core; x replicated (fed k-major). No collectives.

Host prep (pure layout/bit repack, no arithmetic):
  - x -> x.T [K, M] f32 (k-major replica).
  - wq (int8-valued int32) -> int8, transposed+swizzled to [nt*128, kt, n] so each
    n-tile's stationary block DMAs as contiguous 4KB partition lines.
  - scale/bias -> [128, 16] (partition-major per n-tile).

Per-core dataflow (all HWDGE, no SWDGE cast path, no on-chip transposes):
  - x: f32 DMA on the ACT HWDGE ring (its own ring, fine-grained first pieces so
    the first k-tile lands ~10.5us) -> DVE cast f32->bf16 into resident
    xT[kp, kt, m] tiles (contraction on partitions).
  - W: int8 DMA on the SP HWDGE ring per n-tile -> DVE cast int8->bf16 (int8
    values exact in bf16); first 4 tiles cast in two pieces (kt 0..7 / 8..31)
    so the PE's first stationaries are ready early.
  - ~12 dummy warm-up matmuls on a memset tile run during the initial DMA dead
    time so the PE_HAM clock-gate is at 8/8 when real matmuls start.
  - Startup phase: first 4 n-tiles processed k-group-major with 8 live PSUM
    accumulators while x streams in; steady phase: remaining 12 n-tiles k-inner,
    ch-inner (one stationary per (nt,kt) feeds both 512-token chunks).
  - PSUM evicted via one DVE tensor_scalar (x*scale + bias, per-partition
    scalars); outputs stored as out.T f32 on the SP ring.
  - host: concat core outputs along n, transpose to [1024, 16384].
"""

import numpy as np

import concourse.tile as tile
from concourse import bacc, mybir
from concourse.bass_utils import run_bass_kernel_spmd

B, S, K, N = 2, 512, 4096, 16384
M = B * S              # 1024 tokens
NCORES = 8
NSH = N // NCORES      # 2048 out-features per core
P = 128
KT = K // P            # 32 k-tiles
NT = NSH // P          # 16 n-tiles per core
MCW = 512              # moving free dim per matmul (= one PSUM bank of f32)
MCH = M // MCW         # 2 token chunks
NT_A = 4               # n-tiles processed in the k-group-major startup phase
WSPL = 8               # first-phase W tiles cast in (kt<WSPL, kt>=WSPL) pieces
NDUMMY = 12            # warm-up matmuls

# x load piece sizes in k-tiles: small first pieces for fast PE start.
KGS = [1] * 6 + [2] * 13
assert sum(KGS) == KT
KG_START = np.cumsum([0] + KGS).tolist()   # group -> first kt
XG = len(KGS)


def _group_of(kt):
    for g in range(XG):
        if KG_START[g] <= kt < KG_START[g + 1]:
            return g, kt - KG_START[g]
    raise AssertionError


def build(w_bufs: int = 4, x_bufs: int = 4, psum_bufs: int = 8):
    nc = bacc.Bacc("TRN2", target_bir_lowering=False, debug=False)
    xT_d = nc.dram_tensor("xT", [K, M], mybir.dt.float32, kind="ExternalInput")
    w_d = nc.dram_tensor("wq", [NT * P, KT, P], mybir.dt.int8, kind="ExternalInput")
    s_d = nc.dram_tensor("scale", [P, NT], mybir.dt.float32, kind="ExternalInput")
    b_d = nc.dram_tensor("bias", [P, NT], mybir.dt.float32, kind="ExternalInput")
    o_d = nc.dram_tensor("outT", [NSH, M], mybir.dt.float32, kind="ExternalOutput")

    with tile.TileContext(nc) as tc:
        with (
            tc.tile_pool(name="xT_pool", bufs=1) as xT_pool,
            tc.tile_pool(name="xstage", bufs=x_bufs) as xstage_pool,
            tc.tile_pool(name="w8", bufs=w_bufs) as w8_pool,
            tc.tile_pool(name="wT_pool", bufs=w_bufs) as wT_pool,
            tc.tile_pool(name="wTa_pool", bufs=1) as wTa_pool,
            tc.tile_pool(name="wTb_pool", bufs=1) as wTb_pool,
            tc.tile_pool(name="small", bufs=2) as small_pool,
            tc.tile_pool(name="osb", bufs=4) as osb_pool,
            tc.tile_pool(name="psum", bufs=psum_bufs, space="PSUM") as psum_pool,
        ):
            # ---- PE warm-up: dummy matmuls on a zeroed tile during DMA dead time
            dummy = small_pool.tile([P, MCW], mybir.dt.bfloat16, tag="dummy")
            nc.vector.memset(dummy[:], 0.0)

            psA = [
                [
                    psum_pool.tile(
                        [P, MCW], mybir.dt.float32, name=f"psA{nt}_{c}", tag="ps"
                    )
                    for c in range(MCH)
                ]
                for nt in range(NT_A)
            ]
            for i in range(NDUMMY):
                nc.tensor.matmul(
                    psA[0][0][:], dummy[:, 0:P], dummy[:], start=True, stop=True
                )

            # ---- startup DMAs.
            # SP ring: W kt0..WSPL pieces for the first NT_A n-tiles, then the
            # remainders.  ACT ring: x, 1 k-tile at a time (kt0 in two halves).
            w8s = {}
            for nt in range(NT_A):
                w8s[nt] = w8_pool.tile(
                    [P, KT, P], mybir.dt.int8, name=f"w8_{nt}", tag="w8"
                )
                nc.sync.dma_start(
                    out=w8s[nt][:], in_=w_d.ap()[nt * P:(nt + 1) * P]
                )
            xstgs = []
            for g in range(XG):
                xstg = xstage_pool.tile(
                    [P, KGS[g], M], mybir.dt.float32, name=f"xstg{g}", tag="xstg"
                )
                nc.scalar.dma_start(
                    out=xstg[:],
                    in_=xT_d.ap()[
                        KG_START[g] * P:KG_START[g + 1] * P, :
                    ].rearrange("(kt p) m -> p kt m", p=P),
                )
                xstgs.append(xstg)
            s_sb = small_pool.tile([P, NT], mybir.dt.float32, tag="s_sb")
            nc.sync.dma_start(out=s_sb[:], in_=s_d.ap())
            b_sb = small_pool.tile([P, NT], mybir.dt.float32, tag="b_sb")
            nc.sync.dma_start(out=b_sb[:], in_=b_d.ap())

            # ---- DVE cast order: W a-pieces and first x pieces interleaved so
            # neither blocks the other's earliest consumer.
            wTa = {}
            wTb = {}
            xTs = [None] * XG

            def cast_x(g):
                xt = xT_pool.tile(
                    [P, KGS[g], M], mybir.dt.bfloat16, name=f"xT{g}", tag=f"xT{g}"
                )
                nc.vector.tensor_copy(out=xt[:], in_=xstgs[g][:])
                xTs[g] = xt

            for nt in range(NT_A):
                wTa[nt] = wTa_pool.tile(
                    [P, WSPL, P], mybir.dt.bfloat16, name=f"wTa{nt}", tag=f"wTa{nt}"
                )
                nc.vector.tensor_copy(out=wTa[nt][:], in_=w8s[nt][:, 0:WSPL, :])
                cast_x(nt)
            for nt in range(NT_A):
                wTb[nt] = wTb_pool.tile(
                    [P, KT - WSPL, P], mybir.dt.bfloat16, name=f"wTb{nt}",
                    tag=f"wTb{nt}"
                )
                nc.vector.tensor_copy(out=wTb[nt][:], in_=w8s[nt][:, WSPL:KT, :])
                cast_x(NT_A + nt)
            for g in range(2 * NT_A, XG):
                cast_x(g)

            def stationary(nt, kt):
                if nt < NT_A:
                    if kt < WSPL:
                        return wTa[nt][:, kt, :]
                    return wTb[nt][:, kt - WSPL, :]
                return wTs[nt][:, kt, :]

            # ---- phase B W prefetch (nt NT_A..NT_A+3): DMA now, cast before
            # the phase-A evicts enter the DVE queue (in-order engine).
            wTs = {}

            def load_w_full(nt):
                w8 = w8_pool.tile([P, KT, P], mybir.dt.int8, name=f"w8_{nt}", tag="w8")
                nc.sync.dma_start(out=w8[:], in_=w_d.ap()[nt * P:(nt + 1) * P])
                wT = wT_pool.tile(
                    [P, KT, P], mybir.dt.bfloat16, name=f"wT{nt}", tag="wT"
                )
                nc.vector.tensor_copy(out=wT[:], in_=w8[:])
                wTs[nt] = wT

            for nt in range(NT_A, min(NT_A + 4, NT)):
                load_w_full(nt)

            # ---- phase A matmuls: k-group-major across NT_A n-tiles
            for g in range(XG):
                for nt in range(NT_A):
                    for kti in range(KGS[g]):
                        kt = KG_START[g] + kti
                        for c in range(MCH):
                            nc.tensor.matmul(
                                psA[nt][c][:],
                                stationary(nt, kt),
                                xTs[g][:, kti, c * MCW:(c + 1) * MCW],
                                start=(kt == 0),
                                stop=(kt == KT - 1),
                            )

            def evict(nt, c, ps):
                o_sb = osb_pool.tile(
                    [P, MCW], mybir.dt.float32, name=f"osb{nt}_{c}", tag="o_sb"
                )
                nc.vector.tensor_scalar(
                    out=o_sb[:],
                    in0=ps[:],
                    scalar1=s_sb[:, nt:nt + 1],
                    scalar2=b_sb[:, nt:nt + 1],
                    op0=mybir.AluOpType.mult,
                    op1=mybir.AluOpType.add,
                )
                nc.scalar.dma_start(
                    out=o_d.ap()[nt * P:(nt + 1) * P, c * MCW:(c + 1) * MCW],
                    in_=o_sb[:],
                )

            for nt in range(NT_A):
                for c in range(MCH):
                    evict(nt, c, psA[nt][c])

            # ---- phase B: remaining n-tiles, k-inner / ch-inner
            for nt in range(NT_A, NT):
                ps = [
                    psum_pool.tile(
                        [P, MCW], mybir.dt.float32, name=f"ps{nt}_{c}", tag="ps"
                    )
                    for c in range(MCH)
                ]
                for kt in range(KT):
                    g, kti = _group_of(kt)
                    for c in range(MCH):
                        nc.tensor.matmul(
                            ps[c][:],
                            wTs[nt][:, kt, :],
                            xTs[g][:, kti, c * MCW:(c + 1) * MCW],
                            start=(kt == 0),
                            stop=(kt == KT - 1),
                        )
                if nt + 4 < NT:
                    load_w_full(nt + 4)
                for c in range(MCH):
                    evict(nt, c, ps[c])

    nc.compile()
    return nc


def make_in_maps(x, weight_quant, scale, bias):
    x2T = np.ascontiguousarray(
        np.asarray(x, dtype=np.float32).reshape(M, K).T
    )  # [K, M] k-major replica
    wq = np.asarray(weight_quant, dtype=np.int32).astype(np.int8)  # int8-valued
    scale = np.asarray(scale, dtype=np.float32)
    bias = np.asarray(bias, dtype=np.float32)
    in_maps = []
    for i in range(NCORES):
        sl = slice(i * NSH, (i + 1) * NSH)
        # [nsh, k] -> [nt, n, kt, p] -> [nt, p, kt, n] -> [nt*p, kt, n]
        w_sw = np.ascontiguousarray(
            wq[sl].reshape(NT, P, KT, P).transpose(0, 3, 2, 1)
        ).reshape(NT * P, KT, P)
        in_maps.append({
            "xT": x2T,
            "wq": w_sw,
            "scale": np.ascontiguousarray(scale[sl].reshape(NT, P).T),
            "bias": np.ascontiguousarray(bias[sl].reshape(NT, P).T),
        })
    return in_maps


def gather_output(results):
    outT = np.concatenate([np.asarray(r["outT"]) for r in results], axis=0)  # [N, M]
    return np.ascontiguousarray(outT.T).reshape(B, S, N).astype(np.float32, copy=False)


def kernel(x, weight_quant, scale, bias):
    nc = build()
    in_maps = make_in_maps(x, weight_quant, scale, bias)
    res = run_bass_kernel_spmd(nc, in_maps, core_ids=list(range(NCORES)))
    return gather_output(res.results)


if __name__ == "__main__":
    rng = np.random.default_rng(0)
    x = rng.standard_normal((B, S, K), dtype=np.float32)
    wq = rng.integers(-128, 128, size=(N, K), dtype=np.int64).astype(np.int32)
    scale = rng.uniform(0.001, 0.02, size=(N,)).astype(np.float32)
    bias = rng.standard_normal((N,), dtype=np.float32)
    out = kernel(x=x, weight_quant=wq, scale=scale, bias=bias)
    w = wq.astype(np.float32) * scale[:, None]
    exp = x.reshape(M, K) @ w.T + bias
    err = np.abs(out.reshape(M, N) - exp).max() / np.abs(exp).max()
    print("self-check rel err:", err)
